# revision 1
# baseline (speedup 1.0000x reference)
"""Trainium2 Bass kernel for nn_DWT_Layer: 3-level 2D db4 DWT (symmetric mode).

Input  x: (16, 3, 1024, 1024) fp32.
Output:   (16, 3, 64, 128, 128) fp32 — the flattened/truncated wavelet pyramid
          [cA3, cH3, cV3, cD3, cH2, cV2, cD2, cH1, cV1, cD1(truncated)].

Sharding: pure data parallel — the 48 (batch*channel) images are split 6 per
NeuronCore across 8 cores; no communication.

Per-core dataflow, per image, per level (N -> N' = floor((N+5)/2)+1):
  1. width pass along the free axis: 8-tap stride-2 MAC chains on the
     vector (DVE) + gpsimd engines over a symmetric-extension buffer
     (ext built by DMA/copies writing the interior + 4 mirror copies).
  2. height pass as banded fp32 matmuls on the tensor engine: stacked
     [lo; hi] folded DWT matrix (symmetric fold absorbed into weights),
     contraction over partitions; only nonzero 128x128 blocks are run.
  3. scalar (ACT) engine copies PSUM -> SBUF, splitting quadrants; the
     aa quadrant lands in the next level's ext buffer, detail quadrants
     land in per-slot staging tiles that stream to DRAM.
Everything comes out h-major so output DMAs are contiguous-row writes.
"""
import numpy as np

# ----------------------------------------------------------------- constants
DEC_LO = np.array([-0.010597401784997278, 0.032883011666982945,
                   0.030841381835986965, -0.18703481171888114,
                   -0.027983769416983849, 0.63088076792959036,
                   0.71484657055254153, 0.23037781330885523], dtype=np.float64)
L = 8
DEC_HI = np.array([(-1.0) ** (k + 1) * DEC_LO[L - 1 - k] for k in range(L)],
                  dtype=np.float64)
FREV_LO = [float(v) for v in DEC_LO[::-1].astype(np.float32)]
FREV_HI = [float(v) for v in DEC_HI[::-1].astype(np.float32)]
TAPS_ARR = np.tile(np.array(FREV_LO + FREV_HI, dtype=np.float32)[None, :],
                   (128, 1))

B, C, H, W = 16, 3, 1024, 1024
N_CORES = 8
IMGS_PER_CORE = 6
IMG_ELEMS = H * W

LEVELS = [  # (N, N', n_slots_in, n_out_tiles)
    (1024, 515, 8, 9),
    (515, 261, 5, 5),
    (261, 134, 3, 3),
]

# output section offsets (elements within one image's 1048576-long output)
SECT = {}
_cur = 0
for _name, _n in [("cA3", 134), ("cH3", 134), ("cV3", 134), ("cD3", 134),
                  ("cH2", 261), ("cV2", 261), ("cD2", 261),
                  ("cH1", 515), ("cV1", 515), ("cD1", 515)]:
    SECT[_name] = (_cur, _n)
    _cur += _n * _n
# cD1 truncation: keep first 469 full rows + 404 elems of row 469
CD1_FULL_ROWS = 469
CD1_PART_COLS = 404
assert SECT["cD1"][0] + CD1_FULL_ROWS * 515 + CD1_PART_COLS == IMG_ELEMS


def nprime(N):
    return (N + 5) // 2 + 1


def ext_index(j, N):
    if j < 6:
        return 5 - j
    if j < N + 6:
        return j - 6
    return 2 * N + 5 - j


def dwt_matrix(N, filt):
    Np = nprime(N)
    M = np.zeros((Np, N), dtype=np.float64)
    filtrev = filt[::-1]
    for i in range(Np):
        for t in range(L):
            M[i, ext_index(2 * i + t, N)] += filtrev[t]
    return M


def hi_off(Np):
    """row offset of the hi section, padded to a multiple of 32 so that
    engine ops on the hi quadrant start at partition 32/64/0 (BIR verifier:
    SBUF engine APs must start at partition 0/32/64/96)."""
    return ((Np + 31) // 32) * 32


def stacked_matrix(N):
    Np = nprime(N)
    off = hi_off(Np)
    M2 = np.zeros((off + Np, N), dtype=np.float64)
    M2[0:Np] = dwt_matrix(N, DEC_LO)
    M2[off:] = dwt_matrix(N, DEC_HI)
    return M2.astype(np.float32)


def band_blocks(N):
    """[(t, q, kq, mt, band_pos)]: nonzero blocks of M2^T; band_pos tags
    first/last per (t) for start/stop flags."""
    M2 = stacked_matrix(N)
    R = M2.shape[0]
    kt = (N + 127) // 128
    ot = (R + 127) // 128
    per_t = []
    for t in range(ot):
        qs = []
        for q in range(kt):
            blk = M2[t * 128:(t + 1) * 128, q * 128:(q + 1) * 128]
            if np.any(blk != 0):
                qs.append(q)
        per_t.append(qs)
    return per_t, kt, ot, R


def const_weights(N):
    """packed lhsT blocks [128, nblocks, 128] + index map {(t,q): b}."""
    M2 = stacked_matrix(N)
    per_t, kt, ot, R = band_blocks(N)
    blocks = [(t, q) for t in range(ot) for q in per_t[t]]
    arr = np.zeros((128, len(blocks), 128), dtype=np.float32)
    idx = {}
    for b, (t, q) in enumerate(blocks):
        blk = M2[t * 128:(t + 1) * 128, q * 128:(q + 1) * 128]  # [mt, kq]
        arr[:blk.shape[1], b, :blk.shape[0]] = blk.T
        idx[(t, q)] = b
    return arr, idx, per_t


WC = {N: const_weights(N) for N, _, _, _ in LEVELS}

# ---- MAC pass tuning knobs ----
GP_FRAC = 0.0     # fraction of width-pass columns offloaded to gpsimd (Pool)
MAC_SPLIT = 1     # independent DVE chains per filter (hides RAW ack latency)
TAP0_ACT = True   # first tap (overwrite mul) on the scalar engine
MIRROR_GP = True  # mirror/memset ops on gpsimd instead of DVE
TAP_MAJOR = True  # emit MAC ops tap-major (interleave chains) vs unit-major
EXT1_BUFS = 3
WB1_BUFS = 3
EXT2_BUFS = 1
WB2_BUFS = 1
DET_BUFS = 6

_BUILT = None  # cached (nc, meta)


def _free_chunks(Np):
    """quadrant-aligned free chunks of <=512: [(c0, c1), ...] covering
    [0:2Np). Each chunk lies in one quadrant and fits one PSUM bank."""
    out = []
    for base in (0, Np):
        c = 0
        while c < Np:
            e = min(c + 512, Np)
            out.append((base + c, base + e))
            c = e
    return out


def _emit_mirror_ops(nc, ext, S, N):
    """Fill ext cols [0:6) and [N+6:N+13) from the interior [6:N+6)."""
    eng = nc.gpsimd if MIRROR_GP else nc.vector
    # left: ext[j] = x[5-j] = ext[6 + 5 - j] -> reversed slice of cols [6:12)
    eng.tensor_copy(out=ext[:, 0:S, 0:6], in_=ext[:, 0:S, 11:5:-1])
    # right: ext[N+6+k] = x[N-1-k] at ext col N+5-k -> reversed (N+5 .. N-1)
    eng.tensor_copy(out=ext[:, 0:S, N + 6:N + 13],
                    in_=ext[:, 0:S, N + 5:N - 2:-1])


def _emit_mac_pass(nc, ext, wb, S, N, Np, taps_sb, tmp_pool, lvl):
    """width pass: wb[:, s, c + base] = sum_t frev[t] * ext[:, s, 2c+t].

    DVE runs fused multiply-accumulate (scalar_tensor_tensor) chains;
    a GP_FRAC column share goes to gpsimd as mult+add pairs (walrus
    rejects TensorScalarPtr on Pool). Ops are emitted tap-major so
    independent chains interleave and hide the RAW pipeline latency."""
    import concourse.mybir as mybir
    gp_n = int(Np * GP_FRAC)
    dve_n = Np - gp_n
    units = []  # (kind, fi, c0, c1, tmp)
    nsub = max(1, MAC_SPLIT)
    bounds = [round(dve_n * i / nsub) for i in range(nsub + 1)]
    for fi in (0, 1):
        for si in range(nsub):
            if bounds[si] < bounds[si + 1]:
                units.append(("v", fi, bounds[si], bounds[si + 1], None))
        if gp_n > 0:
            tmp = tmp_pool.tile([128, S, gp_n], mybir.dt.float32,
                                tag=f"gtmp{lvl}", bufs=3,
                                name=f"gtmp{lvl}_{fi}")
            units.append(("g", fi, dve_n, Np, tmp))

    order = ([(t, u) for t in range(L) for u in units] if TAP_MAJOR
             else [(t, u) for u in units for t in range(L)])
    for t, u in order:
        if True:
            kind, fi, c0, c1, tmp = u
            frev = FREV_LO if fi == 0 else FREV_HI
            n = c1 - c0
            base = fi * Np
            src = ext[:, 0:S, 2 * c0 + t: 2 * c0 + t + 2 * (n - 1) + 1: 2]
            dst = wb[:, 0:S, base + c0: base + c1]
            if kind == "v":
                if t == 0:
                    if TAP0_ACT:
                        nc.scalar.mul(dst, src, frev[t])
                    else:
                        nc.vector.tensor_scalar_mul(dst, src, frev[t])
                else:
                    nc.vector.scalar_tensor_tensor(
                        out=dst, in0=src, scalar=frev[t], in1=dst,
                        op0=mybir.AluOpType.mult, op1=mybir.AluOpType.add)
            else:
                btap = taps_sb[:, fi * 8 + t:fi * 8 + t + 1].to_broadcast(
                    (128, S, n))
                if t == 0:
                    nc.gpsimd.tensor_tensor(out=dst, in0=src, in1=btap,
                                            op=mybir.AluOpType.mult)
                else:
                    nc.gpsimd.tensor_tensor(out=tmp[:, 0:S, 0:n], in0=src,
                                            in1=btap, op=mybir.AluOpType.mult)
                    nc.gpsimd.tensor_tensor(out=dst, in0=dst,
                                            in1=tmp[:, 0:S, 0:n],
                                            op=mybir.AluOpType.add)


def build_bass(n_images=IMGS_PER_CORE, repeats=1):
    import concourse.mybir as mybir
    import concourse.tile as tile
    from concourse import bacc
    from contextlib import ExitStack

    nc = bacc.Bacc("TRN2", target_bir_lowering=False, debug=False)

    xin = nc.dram_tensor("xin", (n_images, H, W), mybir.dt.float32,
                         kind="ExternalInput").ap()
    out = nc.dram_tensor("out", (n_images, IMG_ELEMS), mybir.dt.float32,
                         kind="ExternalOutput").ap()
    wdram = {}
    for N, _, _, _ in LEVELS:
        arr, _, _ = WC[N]
        wdram[N] = nc.dram_tensor(f"w{N}", arr.shape, mybir.dt.float32,
                                  kind="ExternalInput").ap()
    taps_dram = nc.dram_tensor("taps", (128, 16), mybir.dt.float32,
                               kind="ExternalInput").ap()

    with tile.TileContext(nc) as tc, ExitStack() as ctx:
        cpool = ctx.enter_context(tc.tile_pool(name="consts", bufs=1))
        extp = ctx.enter_context(tc.tile_pool(name="ext", bufs=1))
        wbp = ctx.enter_context(tc.tile_pool(name="wb", bufs=1))
        psp = ctx.enter_context(tc.tile_pool(name="ps", bufs=1, space="PSUM"))
        detp = ctx.enter_context(tc.tile_pool(name="det", bufs=1))

        wsb = {}
        for N, _, _, _ in LEVELS:
            arr, _, _ = WC[N]
            wsb[N] = cpool.tile(list(arr.shape), mybir.dt.float32,
                                name=f"wsb{N}")
            nc.sync.dma_start(out=wsb[N][:], in_=wdram[N])
        taps_sb = cpool.tile([128, 16], mybir.dt.float32, name="taps_sb")
        nc.sync.dma_start(out=taps_sb[:], in_=taps_dram)

        for _rep in range(repeats):
            for img in range(n_images):
                _emit_image(nc, tc, extp, wbp, psp, detp, wsb, taps_sb,
                            xin, out, img)

    nc.compile()
    return nc


def _emit_image(nc, tc, extp, wbp, psp, detp, wsb, taps_sb, xin, out, img):
    import concourse.mybir as mybir

    N1, P1 = 1024, 515
    # ---------------- L1: ext halves + MACs ----------------
    halves = []
    for h in range(2):
        ext = extp.tile([128, 4, N1 + 13], mybir.dt.float32, tag="ext1",
                        bufs=EXT1_BUFS, name=f"ext1_{img}_{h}")
        src = xin[img, 512 * h:512 * (h + 1), :].rearrange(
            "(s p) w -> p s w", p=128)
        nc.sync.dma_start(out=ext[:, 0:4, 6:N1 + 6], in_=src)
        _emit_mirror_ops(nc, ext, 4, N1)
        wb = wbp.tile([128, 4, 2 * P1], mybir.dt.float32, tag="wb1",
                      bufs=WB1_BUFS, name=f"wb1_{img}_{h}")
        _emit_mac_pass(nc, ext, wb, 4, N1, P1, taps_sb, wbp, 1)
        halves.append(wb)

    def rhs1(q, c0, c1):
        return halves[q // 4][:, q % 4, c0:c1]

    # next-level ext buffers; memset the partial last slot so the unwritten
    # partitions (beyond the valid rows) are finite zeros
    ext2 = extp.tile([128, 5, 515 + 13], mybir.dt.float32, tag="ext2",
                     bufs=EXT2_BUFS, name=f"ext2_{img}")
    (nc.gpsimd if MIRROR_GP else nc.vector).memset(ext2[:, 4, :], 0.0)
    ext3 = extp.tile([128, 3, 261 + 13], mybir.dt.float32, tag="ext3",
                     bufs=EXT2_BUFS, name=f"ext3_{img}")
    (nc.gpsimd if MIRROR_GP else nc.vector).memset(ext3[:, 2, :], 0.0)

    _emit_level_mm(nc, psp, detp, wsb, out, img, N=1024, rhs=rhs1,
                   next_ext=ext2, det_names=("cH1", "cV1", "cD1"))
    _emit_mirror_ops(nc, ext2, 5, 515)

    wb2 = wbp.tile([128, 5, 2 * 261], mybir.dt.float32, tag="wb2",
                   bufs=WB2_BUFS, name=f"wb2_{img}")
    _emit_mac_pass(nc, ext2, wb2, 5, 515, 261, taps_sb, wbp, 2)

    def rhs2(q, c0, c1):
        return wb2[:, q, c0:c1]

    _emit_level_mm(nc, psp, detp, wsb, out, img, N=515, rhs=rhs2,
                   next_ext=ext3, det_names=("cH2", "cV2", "cD2"))
    _emit_mirror_ops(nc, ext3, 3, 261)

    wb3 = wbp.tile([128, 3, 2 * 134], mybir.dt.float32, tag="wb3",
                   bufs=WB2_BUFS, name=f"wb3_{img}")
    _emit_mac_pass(nc, ext3, wb3, 3, 261, 134, taps_sb, wbp, 3)

    def rhs3(q, c0, c1):
        return wb3[:, q, c0:c1]

    _emit_level_mm(nc, psp, detp, wsb, out, img, N=261, rhs=rhs3,
                   next_ext=None, det_names=("cH3", "cV3", "cD3"))


def _emit_level_mm(nc, psp, detp, wsb, out, img, N, rhs, next_ext, det_names):
    """height-pass matmuls + psum->sbuf quadrant copies + detail DMAs."""
    import concourse.mybir as mybir

    Np = nprime(N)
    arr, idx, per_t = WC[N]
    OFF = hi_off(Np)
    R = OFF + Np
    ot = (R + 127) // 128
    kN = N  # contraction length
    chunks = _free_chunks(Np)

    for t in range(ot):
        mt = min(128, R - t * 128)
        qs = per_t[t]
        ps_tiles = []
        for ci, (c0, c1) in enumerate(chunks):
            w = c1 - c0
            tag = "psA" if w > 256 else "psB"
            ps = psp.tile([128, w], mybir.dt.float32, tag=tag, bufs=4,
                          name=f"ps_{img}_{N}_{t}_{ci}")
            ps_tiles.append(ps)
            for ki, q in enumerate(qs):
                kq = min(128, kN - q * 128)
                r = rhs(q, c0, c1)
                if kq < 128:
                    r = r[0:kq]
                nc.tensor.matmul(
                    ps[0:mt, 0:w],
                    wsb[N][0:kq, idx[(t, q)], 0:mt],
                    r,
                    start=(ki == 0), stop=(ki == len(qs) - 1))

        # quadrant qd -> list of (ps_tile, dst_col0, width)
        quad_srcs = {0: [], 1: []}
        for ci, (c0, c1) in enumerate(chunks):
            qd = 0 if c0 < Np else 1
            quad_srcs[qd].append((ps_tiles[ci], c0 - qd * Np, c1 - c0))

        # lo rows: global [0:Np); hi rows: global [OFF:OFF+Np)
        lo_end = min(128, Np - t * 128) if t * 128 < Np else 0
        hp0 = max(0, OFF - t * 128)
        hp1 = max(0, min(128, OFF + Np - t * 128))
        # split hi ranges at legal partition starts (0/32/64)
        hi_ranges = []
        if hp0 < hp1:
            if hp0 == 0:
                hi_ranges = [(0, hp1)]
            else:
                assert hp0 == 32, hp0
                hi_ranges = [(32, min(64, hp1))]
                if hp1 > 64:
                    hi_ranges.append((64, hp1))

        if lo_end > 0:
            # quadrant 0 = aa -> next level ext (or cA3 staging tile)
            if next_ext is not None:
                for ps, d0, w in quad_srcs[0]:
                    nc.scalar.copy(out=next_ext[0:lo_end, t, 6 + d0:6 + d0 + w],
                                   in_=ps[0:lo_end, 0:w])
            else:
                _emit_det_copy_dma(nc, detp, out, img, "cA3", Np,
                                   quad_srcs[0], t, [(0, lo_end)], 0)
            # quadrant 1 = ad = cV
            _emit_det_copy_dma(nc, detp, out, img, det_names[1], Np,
                               quad_srcs[1], t, [(0, lo_end)], 0)
        if hi_ranges:
            # hi rows: da = cH (quadrant 0), dd = cD (quadrant 1)
            _emit_det_copy_dma(nc, detp, out, img, det_names[0], Np,
                               quad_srcs[0], t, hi_ranges, OFF)
            _emit_det_copy_dma(nc, detp, out, img, det_names[2], Np,
                               quad_srcs[1], t, hi_ranges, OFF)


def _emit_det_copy_dma(nc, detp, out, img, sec_name, Np, srcs, t, pranges,
                       row_off):
    """Copy psum chunks into a staging tile, then DMA rows to DRAM.

    h (row index within the detail) = 128*t + p - row_off for partition p.
    pranges: list of legal-start partition ranges covering this tile's rows."""
    import concourse.mybir as mybir
    sec_base, Wd = SECT[sec_name]
    assert Wd == Np
    p0, p1 = pranges[0][0], pranges[-1][1]
    h0 = 128 * t + p0 - row_off
    h1 = h0 + (p1 - p0)
    assert 0 <= h0 and h1 <= Np, (sec_name, t, pranges, h0, h1)

    is_cd1 = sec_name == "cD1"
    if is_cd1 and h0 >= CD1_FULL_ROWS + 1:
        return  # fully truncated
    dt = detp.tile([128, Np], mybir.dt.float32, tag=f"det{Np}", bufs=DET_BUFS,
                   name=f"det_{sec_name}_{img}_{t}_{p0}")
    for ps, d0, w in srcs:
        for (a, b) in pranges:
            nc.scalar.copy(out=dt[a:b, d0:d0 + w], in_=ps[a:b, 0:w])

    full_h1 = h1
    if is_cd1 and h1 > CD1_FULL_ROWS:
        full_h1 = CD1_FULL_ROWS
    if full_h1 > h0:
        npart = full_h1 - h0
        dst = out[img, sec_base + h0 * Wd: sec_base + full_h1 * Wd].rearrange(
            "(h w) -> h w", w=Wd)
        nc.sync.dma_start(out=dst, in_=dt[p0:p0 + npart, :])
    if is_cd1 and h0 <= CD1_FULL_ROWS < h1:
        pp = p0 + (CD1_FULL_ROWS - h0)
        dst = out[img, sec_base + CD1_FULL_ROWS * Wd:
                  sec_base + CD1_FULL_ROWS * Wd + CD1_PART_COLS]
        nc.sync.dma_start(out=dst.rearrange("(h w) -> h w", w=CD1_PART_COLS),
                          in_=dt[pp:pp + 1, 0:CD1_PART_COLS])


# ----------------------------------------------------------------- runner
def _get_built():
    global _BUILT
    if _BUILT is None:
        _BUILT = build_bass()
    return _BUILT


def kernel(x: np.ndarray) -> np.ndarray:
    from concourse import bass_utils

    x = np.ascontiguousarray(np.asarray(x), dtype=np.float32)
    assert x.shape == (B, C, H, W), x.shape
    nc = _get_built()

    imgs = x.reshape(B * C, H, W)
    in_maps = []
    for c in range(N_CORES):
        m = {"xin": imgs[c * IMGS_PER_CORE:(c + 1) * IMGS_PER_CORE]}
        for N, _, _, _ in LEVELS:
            m[f"w{N}"] = WC[N][0]
        m["taps"] = TAPS_ARR
        in_maps.append(m)

    res = bass_utils.run_bass_kernel_spmd(nc, in_maps,
                                          core_ids=list(range(N_CORES)))
    outs = [res.results[c]["out"] for c in range(N_CORES)]
    flat = np.concatenate(outs, axis=0)  # [48, 1048576]
    return flat.reshape(B, C, 64, 128, 128)



# revision 6
# speedup vs baseline: 5.2738x; 5.2738x over previous
"""Trainium2 Bass kernel for nn_DWT_Layer: 3-level 2D db4 DWT (symmetric mode).

Input  x: (16, 3, 1024, 1024) fp32.
Output:   (16, 3, 64, 128, 128) fp32 — the flattened/truncated wavelet pyramid
          [cA3, cH3, cV3, cD3, cH2, cV2, cD2, cH1, cV1, cD1(truncated)].

Sharding: pure data parallel — 48 (batch*channel) images, 6 per core on 8
NeuronCores, no communication.

Algorithm (all-PE, transpose-free): each 1D DWT pass along the partition
axis is a set of banded fp16 matmuls out[m,j] = sum_r A[r,m]*M2[j,r] with
the DATA as lhsT and the folded/stacked DWT band matrix as rhs. The
contraction rows are stored in overlapping 128-row "slots" (stride <=122)
so that every output row j is owned by exactly one slot -> each psum
column is written by a single start=stop matmul (no accumulation, no
pre-zeroing) and the output comes out transposed. Running the same pass
twice (height then width) returns to row-major orientation, so the whole
3-level pyramid needs zero transposes, zero DVE MAC chains and zero
mirror ops: just cast-DMAs in, banded matmuls, PSUM->SBUF copies
(fp32->fp16 for the next stage / fp32 for detail staging) and row DMAs
out.
"""
import numpy as np

# ----------------------------------------------------------------- constants
DEC_LO = np.array([-0.010597401784997278, 0.032883011666982945,
                   0.030841381835986965, -0.18703481171888114,
                   -0.027983769416983849, 0.63088076792959036,
                   0.71484657055254153, 0.23037781330885523], dtype=np.float64)
L = 8
DEC_HI = np.array([(-1.0) ** (k + 1) * DEC_LO[L - 1 - k] for k in range(L)],
                  dtype=np.float64)

B, C, H, W = 16, 3, 1024, 1024
N_CORES = 8
IMGS_PER_CORE = 6
IMG_ELEMS = H * W

LEVEL_NS = [1024, 515, 261]   # input edge length per level


def nprime(N):
    return (N + 5) // 2 + 1


def ext_index(j, N):
    if j < 6:
        return 5 - j
    if j < N + 6:
        return j - 6
    return 2 * N + 5 - j


def dwt_matrix(N, filt):
    Np = nprime(N)
    M = np.zeros((Np, N), dtype=np.float64)
    filtrev = filt[::-1]
    for i in range(Np):
        for t in range(L):
            M[i, ext_index(2 * i + t, N)] += filtrev[t]
    return M


def build_slots(N):
    """[(o, j0, j1)]: slot covers input rows [o, o+128); owns outputs
    [j0, j1) whose (lo and hi) supports lie inside the slot."""
    Np = nprime(N)
    Mlo = dwt_matrix(N, DEC_LO)
    Mhi = dwt_matrix(N, DEC_HI)
    lo_r, hi_r = [], []
    for j in range(Np):
        nz = np.nonzero(np.abs(Mlo[j]) + np.abs(Mhi[j]))[0]
        lo_r.append(int(nz.min()))
        hi_r.append(int(nz.max()))
    slots = []
    j = 0
    while j < Np:
        o = min(max(0, lo_r[j]), N - 128)
        j1 = j
        while j1 < Np and lo_r[j1] >= o and hi_r[j1] < o + 128:
            j1 += 1
        assert j1 > j
        slots.append((o, j, j1))
        j = j1
    return slots


BW = 64  # max band width per slot


def build_weights(N):
    """fp16 packed rhs bands: [128, n_slots, 2, BW]."""
    slots = build_slots(N)
    Mlo = dwt_matrix(N, DEC_LO)
    Mhi = dwt_matrix(N, DEC_HI)
    arr = np.zeros((128, len(slots), 2, BW), dtype=np.float16)
    for s, (o, j0, j1) in enumerate(slots):
        assert j1 - j0 <= BW
        arr[:, s, 0, :j1 - j0] = Mlo[j0:j1, o:o + 128].T
        arr[:, s, 1, :j1 - j0] = Mhi[j0:j1, o:o + 128].T
    return arr


SLOTS = {N: build_slots(N) for N in LEVEL_NS}
WEIGHTS = {N: build_weights(N) for N in LEVEL_NS}

# p2 m-chunk offsets (within the 2Np free axis of B) per level:
#   lo chunks = next level's A slots (aa feeds them 1:1); hi chunks cover the
#   hi half with non-overlapping 128s + one 128-tail.
P2_LO = {1024: [o for o, _, _ in SLOTS[515]],   # {0,122,244,366,387}
         515: [o for o, _, _ in SLOTS[261]],    # {0,122,133}
         261: [0, 6]}
P2_HI = {1024: [515 + r for r in (0, 128, 256, 384, 387)],
         515: [261 + r for r in (0, 128, 133)],
         261: [134 + r for r in (0, 6)]}

# output section offsets (elements within one image's 1048576-long output)
SECT = {}
_cur = 0
for _name, _n in [("cA3", 134), ("cH3", 134), ("cV3", 134), ("cD3", 134),
                  ("cH2", 261), ("cV2", 261), ("cD2", 261),
                  ("cH1", 515), ("cV1", 515), ("cD1", 515)]:
    SECT[_name] = (_cur, _n)
    _cur += _n * _n
CD1_FULL_ROWS = 469
CD1_PART_COLS = 404
assert SECT["cD1"][0] + CD1_FULL_ROWS * 515 + CD1_PART_COLS == IMG_ELEMS

PS_W = 1536      # psum tile width (bank-aligned: 3 x 512 fp32)
PS_BUFS = 2

_BUILT = None


def _split_512(a, b):
    """split [a, b) at multiples of 512 (psum bank boundaries)."""
    out = []
    while a < b:
        e = min(b, (a // 512 + 1) * 512)
        out.append((a, e))
        a = e
    return out


def _emit_pass(nc, psp, N, lhsT_of, wsb, sink, copy_engines):
    """One DWT pass: for each m-chunk, banded matmuls into a psum tile,
    then sink(ci, ps) emits the copies out of psum."""
    Np = nprime(N)
    slots = SLOTS[N]
    chunks = sink.chunks
    for ci, m0 in enumerate(chunks):
        ps = psp.tile([128, PS_W], _dt().float32, tag="ps", bufs=PS_BUFS,
                      name=f"ps_{sink.tag}_{ci}")
        for si, (o, j0, j1) in enumerate(slots):
            lhsT = lhsT_of(si, m0)
            for f in (0, 1):
                c0, c1 = f * Np + j0, f * Np + j1
                for (a, b) in _split_512(c0, c1):
                    w0 = a - f * Np - j0
                    nc.tensor.matmul(ps[0:128, a:b], lhsT,
                                     wsb[:, si, f, w0:w0 + (b - a)],
                                     start=True, stop=True)
        sink(ci, ps, copy_engines)


def _dt():
    import concourse.mybir as mybir
    return mybir.dt


class _EngRot:
    """round-robin copy chooser, weighted toward the faster engines."""

    def __init__(self, nc):
        def act(out, in_):
            nc.scalar.copy(out=out, in_=in_)

        def dve(out, in_):
            nc.vector.tensor_copy(out=out, in_=in_)

        self.seq = [act, dve]
        self.i = 0

    def next(self):
        e = self.seq[self.i % len(self.seq)]
        self.i += 1
        return e


def build_bass(n_images=IMGS_PER_CORE, repeats=1):
    import concourse.mybir as mybir
    import concourse.tile as tile
    from concourse import bacc
    from concourse.ap import AP
    from contextlib import ExitStack

    nc = bacc.Bacc("TRN2", target_bir_lowering=False, debug=False)

    xin = nc.dram_tensor("xin", (n_images, H, W), mybir.dt.float32,
                         kind="ExternalInput").ap()
    out = nc.dram_tensor("out", (n_images, IMG_ELEMS), mybir.dt.float32,
                         kind="ExternalOutput").ap()
    wdram = {N: nc.dram_tensor(f"w{N}", WEIGHTS[N].shape, mybir.dt.float16,
                               kind="ExternalInput").ap() for N in LEVEL_NS}

    with tile.TileContext(nc) as tc, ExitStack() as ctx:
        cpool = ctx.enter_context(tc.tile_pool(name="consts", bufs=1))
        apool = ctx.enter_context(tc.tile_pool(name="act", bufs=1))
        stpool = ctx.enter_context(tc.tile_pool(name="stage", bufs=1))
        psp = ctx.enter_context(tc.tile_pool(name="ps", bufs=1, space="PSUM"))

        wsb = {}
        for N in LEVEL_NS:
            wsb[N] = cpool.tile(list(WEIGHTS[N].shape), mybir.dt.float16,
                                name=f"wsb{N}")
            nc.sync.dma_start(out=wsb[N][:], in_=wdram[N])

        rot = _EngRot(nc)
        for _rep in range(repeats):
            for img in range(n_images):
                _emit_image(nc, tc, apool, stpool, psp, wsb, xin, out, img,
                            rot, AP)

    nc.compile()
    return nc


class _P1Sink:
    """p1: psum [128, 2Np] -> B[:, ci, :] (fp16)."""

    def __init__(self, nc, Np, Btile, tag):
        self.nc, self.Np, self.B, self.tag = nc, Np, Btile, tag
        self.chunks = [o for o, _, _ in SLOTS[{515: 1024, 261: 515,
                                               134: 261}[Np]]]

    def __call__(self, ci, ps, rot):
        Np = self.Np
        rot.next()(self.B[:, ci, 0:Np], ps[:, 0:Np])
        rot.next()(self.B[:, ci, Np:2 * Np], ps[:, Np:2 * Np])


class _P2Sink:
    """p2: psum -> next-level A (fp16) + fp32 det staging tiles; DMAs are
    emitted by the caller once the stage tiles fill."""

    def __init__(self, nc, N, A_next, st_lo_hi, st_hi_lo, st_hi_hi, st_lo_lo):
        # st_lo_hi = cV stage (lo chunks, hi cols); st_hi_lo = cH;
        # st_hi_hi = cD; st_lo_lo = cA3 stage (only for the last level).
        self.nc = nc
        self.Np = nprime(N)
        self.A_next, self.cv, self.ch, self.cd = A_next, st_lo_hi, st_hi_lo, st_hi_hi
        self.ca = st_lo_lo
        self.lo = P2_LO[N]
        self.hi = P2_HI[N]
        self.chunks = self.lo + self.hi
        self.N = N

    def __call__(self, ci, ps, rot):
        Np = self.Np
        if ci < len(self.lo):
            if self.A_next is not None:
                rot.next()(self.A_next[:, ci, :], ps[:, 0:Np])
            else:
                rot.next()(self.ca[:, ci, :], ps[:, 0:Np])
            rot.next()(self.cv[:, ci, :], ps[:, Np:2 * Np])
        else:
            hc = ci - len(self.lo)
            rot.next()(self.ch[:, hc, :], ps[:, 0:Np])
            if not (self.N == 1024 and hc == len(self.hi) - 1):
                # cD1's last slot is fully covered by slot 3 (truncation)
                rot.next()(self.cd[:, hc, :], ps[:, Np:2 * Np])


def _emit_det_dmas(nc, out, img, name, stage, row_offs, AP):
    """DMA a staged detail section (overlapping slot layout) to DRAM."""
    sec_base, Wd = SECT[name]
    base = img * IMG_ELEMS + sec_base
    n_slots = len(row_offs)
    if name == "cD1":
        # slots rows {0,128,256,384,387}; valid: 469 full rows + 404 cols
        dst = AP(out.tensor, base, [[Wd, 128], [128 * Wd, 3], [1, Wd]])
        nc.sync.dma_start(out=dst, in_=stage[:, 0:3, :])
        dst = AP(out.tensor, base + 384 * Wd, [[Wd, 85], [1, Wd]])
        nc.sync.dma_start(out=dst, in_=stage[0:85, 3, :])
        dst = AP(out.tensor, base + CD1_FULL_ROWS * Wd, [[1, CD1_PART_COLS]])
        nc.sync.dma_start(out=dst, in_=stage[85:86, 3, 0:CD1_PART_COLS])
        return
    if n_slots == 2 and row_offs[1] - row_offs[0] < 128:
        # one overlapping-dest DMA covers everything (L3 sections)
        step = row_offs[1] - row_offs[0]
        dst = AP(out.tensor, base, [[Wd, 128], [step * Wd, 2], [1, Wd]])
        nc.sync.dma_start(out=dst, in_=stage[:, 0:2, :])
        return
    # uniform prefix + fresh tail
    step = row_offs[1] - row_offs[0]
    nu = n_slots - 1
    dst = AP(out.tensor, base, [[Wd, 128], [step * Wd, nu], [1, Wd]])
    nc.sync.dma_start(out=dst, in_=stage[:, 0:nu, :])
    o_last = row_offs[-1]
    fresh0 = row_offs[-2] + 128          # first row not covered by prefix
    p0 = fresh0 - o_last
    npart = o_last + 128 - fresh0
    dst = AP(out.tensor, base + fresh0 * Wd, [[Wd, npart], [1, Wd]])
    nc.sync.dma_start(out=dst, in_=stage[p0:p0 + npart, n_slots - 1, :])


def _emit_image(nc, tc, apool, stpool, psp, wsb, xin, out, img, rot, AP):
    import concourse.mybir as mybir
    f16, f32 = mybir.dt.float16, mybir.dt.float32

    # ---------------- input: cast DMA into overlapping h-slots ----------
    A1 = apool.tile([128, 9, 1024], f16, tag="A1", bufs=2, name=f"A1_{img}")
    src = AP(xin.tensor, img * H * W, [[W, 128], [122 * W, 8], [1, W]])
    nc.gpsimd.dma_start(out=A1[:, 0:8, :], in_=src)
    nc.gpsimd.dma_start(out=A1[:, 8, :], in_=xin[img, 896:1024, :])

    # ---------------- L1 ------------------------------------------------
    B1 = apool.tile([128, 9, 1030], f16, tag="B1", bufs=2, name=f"B1_{img}")
    s1 = _P1Sink(nc, 515, B1, f"p1a_{img}")
    s1.tag = f"p1a_{img}"
    _emit_pass(nc, psp, 1024, lambda si, m0: A1[:, si, m0:m0 + 128],
               wsb[1024], s1, rot)

    A2 = apool.tile([128, 5, 515], f16, tag="A2", bufs=2, name=f"A2_{img}")
    cv1 = stpool.tile([128, 5, 515], f32, tag="st515", bufs=4, name=f"cv1_{img}")
    ch1 = stpool.tile([128, 5, 515], f32, tag="st515", bufs=4, name=f"ch1_{img}")
    cd1 = stpool.tile([128, 5, 515], f32, tag="st515", bufs=4, name=f"cd1_{img}")
    s2 = _P2Sink(nc, 1024, A2, cv1, ch1, cd1, None)
    s2.tag = f"p2a_{img}"
    _emit_pass(nc, psp, 1024, lambda si, m0: B1[:, si, m0:m0 + 128],
               wsb[1024], s2, rot)
    _emit_det_dmas(nc, out, img, "cV1", cv1, [0, 122, 244, 366, 387], AP)
    _emit_det_dmas(nc, out, img, "cH1", ch1, [0, 128, 256, 384, 387], AP)
    _emit_det_dmas(nc, out, img, "cD1", cd1, [0, 128, 256, 384, 387], AP)

    # ---------------- L2 ------------------------------------------------
    B2 = apool.tile([128, 5, 522], f16, tag="B2", bufs=2, name=f"B2_{img}")
    s3 = _P1Sink(nc, 261, B2, f"p1b_{img}")
    s3.tag = f"p1b_{img}"
    _emit_pass(nc, psp, 515, lambda si, m0: A2[:, si, m0:m0 + 128],
               wsb[515], s3, rot)

    A3 = apool.tile([128, 3, 261], f16, tag="A3", bufs=2, name=f"A3_{img}")
    cv2 = stpool.tile([128, 3, 261], f32, tag="st261", bufs=4, name=f"cv2_{img}")
    ch2 = stpool.tile([128, 3, 261], f32, tag="st261", bufs=4, name=f"ch2_{img}")
    cd2 = stpool.tile([128, 3, 261], f32, tag="st261", bufs=4, name=f"cd2_{img}")
    s4 = _P2Sink(nc, 515, A3, cv2, ch2, cd2, None)
    s4.tag = f"p2b_{img}"
    _emit_pass(nc, psp, 515, lambda si, m0: B2[:, si, m0:m0 + 128],
               wsb[515], s4, rot)
    _emit_det_dmas(nc, out, img, "cV2", cv2, [0, 122, 133], AP)
    _emit_det_dmas(nc, out, img, "cH2", ch2, [0, 128, 133], AP)
    _emit_det_dmas(nc, out, img, "cD2", cd2, [0, 128, 133], AP)

    # ---------------- L3 ------------------------------------------------
    B3 = apool.tile([128, 3, 268], f16, tag="B3", bufs=2, name=f"B3_{img}")
    s5 = _P1Sink(nc, 134, B3, f"p1c_{img}")
    s5.tag = f"p1c_{img}"
    _emit_pass(nc, psp, 261, lambda si, m0: A3[:, si, m0:m0 + 128],
               wsb[261], s5, rot)

    ca3 = stpool.tile([128, 2, 134], f32, tag="st134", bufs=8, name=f"ca3_{img}")
    cv3 = stpool.tile([128, 2, 134], f32, tag="st134", bufs=8, name=f"cv3_{img}")
    ch3 = stpool.tile([128, 2, 134], f32, tag="st134", bufs=8, name=f"ch3_{img}")
    cd3 = stpool.tile([128, 2, 134], f32, tag="st134", bufs=8, name=f"cd3_{img}")
    s6 = _P2Sink(nc, 261, None, cv3, ch3, cd3, ca3)
    s6.tag = f"p2c_{img}"
    _emit_pass(nc, psp, 261, lambda si, m0: B3[:, si, m0:m0 + 128],
               wsb[261], s6, rot)
    _emit_det_dmas(nc, out, img, "cA3", ca3, [0, 6], AP)
    _emit_det_dmas(nc, out, img, "cV3", cv3, [0, 6], AP)
    _emit_det_dmas(nc, out, img, "cH3", ch3, [0, 6], AP)
    _emit_det_dmas(nc, out, img, "cD3", cd3, [0, 6], AP)


# ----------------------------------------------------------------- runner
EXTRA_INPUTS = {f"w{N}": WEIGHTS[N] for N in LEVEL_NS}


def _get_built():
    global _BUILT
    if _BUILT is None:
        _BUILT = build_bass()
    return _BUILT


def kernel(x: np.ndarray) -> np.ndarray:
    from concourse import bass_utils

    x = np.ascontiguousarray(np.asarray(x), dtype=np.float32)
    assert x.shape == (B, C, H, W), x.shape
    nc = _get_built()

    imgs = x.reshape(B * C, H, W)
    in_maps = []
    for c in range(N_CORES):
        m = {"xin": imgs[c * IMGS_PER_CORE:(c + 1) * IMGS_PER_CORE]}
        m.update(EXTRA_INPUTS)
        in_maps.append(m)

    res = bass_utils.run_bass_kernel_spmd(nc, in_maps,
                                          core_ids=list(range(N_CORES)))
    outs = [res.results[c]["out"] for c in range(N_CORES)]
    flat = np.concatenate(outs, axis=0)  # [48, 1048576]
    return flat.reshape(B, C, 64, 128, 128)


# revision 17
# speedup vs baseline: 24.4892x; 4.6436x over previous
"""Trainium2 Bass kernel for nn_DWT_Layer: 3-level 2D db4 DWT (symmetric mode).

Input  x: (16, 3, 1024, 1024) fp32.
Output:   (16, 3, 64, 128, 128) fp32 — the flattened/truncated wavelet pyramid
          [cA3, cH3, cV3, cD3, cH2, cV2, cD2, cH1, cV1, cD1(truncated)].

Sharding: pure data parallel — 48 (batch*channel) images, 6 per core on 8
NeuronCores, no communication.

Algorithm (all-PE, transpose-free): each 1D DWT pass along the partition
axis is a set of banded fp16 matmuls out[m,j] = sum_r A[r,m]*M2[j,r] with
the DATA as lhsT and the folded/stacked DWT band matrix as rhs. The
contraction rows are stored in overlapping 128-row "slots" (stride <=122)
so that every output row j is owned by exactly one slot -> each psum
column is written by a single start=stop matmul (no accumulation, no
pre-zeroing) and the output comes out transposed. Running the same pass
twice (height then width) returns to row-major orientation, so the whole
3-level pyramid needs zero transposes, zero DVE MAC chains and zero
mirror ops: just cast-DMAs in, banded matmuls, PSUM->SBUF copies
(fp32->fp16 for the next stage / fp32 for detail staging) and row DMAs
out.
"""
import numpy as np

# ----------------------------------------------------------------- constants
DEC_LO = np.array([-0.010597401784997278, 0.032883011666982945,
                   0.030841381835986965, -0.18703481171888114,
                   -0.027983769416983849, 0.63088076792959036,
                   0.71484657055254153, 0.23037781330885523], dtype=np.float64)
L = 8
DEC_HI = np.array([(-1.0) ** (k + 1) * DEC_LO[L - 1 - k] for k in range(L)],
                  dtype=np.float64)

B, C, H, W = 16, 3, 1024, 1024
N_CORES = 8
IMGS_PER_CORE = 6
IMG_ELEMS = H * W

LEVEL_NS = [1024, 515, 261]   # input edge length per level


def nprime(N):
    return (N + 5) // 2 + 1


def ext_index(j, N):
    if j < 6:
        return 5 - j
    if j < N + 6:
        return j - 6
    return 2 * N + 5 - j


def dwt_matrix(N, filt):
    Np = nprime(N)
    M = np.zeros((Np, N), dtype=np.float64)
    filtrev = filt[::-1]
    for i in range(Np):
        for t in range(L):
            M[i, ext_index(2 * i + t, N)] += filtrev[t]
    return M


def build_slots(N):
    """[(o, j0, j1)]: slot covers input rows [o, o+128); owns outputs
    [j0, j1) whose (lo and hi) supports lie inside the slot."""
    Np = nprime(N)
    Mlo = dwt_matrix(N, DEC_LO)
    Mhi = dwt_matrix(N, DEC_HI)
    lo_r, hi_r = [], []
    for j in range(Np):
        nz = np.nonzero(np.abs(Mlo[j]) + np.abs(Mhi[j]))[0]
        lo_r.append(int(nz.min()))
        hi_r.append(int(nz.max()))
    slots = []
    j = 0
    while j < Np:
        o = min(max(0, lo_r[j]), N - 128)
        j1 = j
        while j1 < Np and lo_r[j1] >= o and hi_r[j1] < o + 128:
            j1 += 1
        assert j1 > j
        slots.append((o, j, j1))
        j = j1
    return slots


BW = 64  # max band width per slot


def build_weights(N):
    """fp16 packed rhs bands: [128, n_slots, 2, BW]."""
    slots = build_slots(N)
    Mlo = dwt_matrix(N, DEC_LO)
    Mhi = dwt_matrix(N, DEC_HI)
    arr = np.zeros((128, len(slots), 2, BW), dtype=np.float16)
    for s, (o, j0, j1) in enumerate(slots):
        assert j1 - j0 <= BW
        arr[:, s, 0, :j1 - j0] = Mlo[j0:j1, o:o + 128].T
        arr[:, s, 1, :j1 - j0] = Mhi[j0:j1, o:o + 128].T
    return arr


SLOTS = {N: build_slots(N) for N in LEVEL_NS}
WEIGHTS = {N: build_weights(N) for N in LEVEL_NS}

# p2 m-chunk offsets (within the 2Np free axis of B) per level:
#   lo chunks = next level's A slots (aa feeds them 1:1); hi chunks cover the
#   hi half with non-overlapping 128s + one 128-tail.
P2_LO = {1024: [o for o, _, _ in SLOTS[515]],   # {0,122,244,366,387}
         515: [o for o, _, _ in SLOTS[261]],    # {0,122,133}
         261: [0, 6]}
P2_HI = {1024: [515 + r for r in (0, 128, 256, 384, 387)],
         515: [261 + r for r in (0, 128, 133)],
         261: [134 + r for r in (0, 6)]}

# output section offsets (elements within one image's 1048576-long output)
SECT = {}
_cur = 0
for _name, _n in [("cA3", 134), ("cH3", 134), ("cV3", 134), ("cD3", 134),
                  ("cH2", 261), ("cV2", 261), ("cD2", 261),
                  ("cH1", 515), ("cV1", 515), ("cD1", 515)]:
    SECT[_name] = (_cur, _n)
    _cur += _n * _n
CD1_FULL_ROWS = 469
CD1_PART_COLS = 404
assert SECT["cD1"][0] + CD1_FULL_ROWS * 515 + CD1_PART_COLS == IMG_ELEMS

PS_W = 1536      # psum tile width (bank-aligned: 3 x 512 fp32)
PS_BUFS = 2

_BUILT = None


def _split_512(a, b):
    """split [a, b) at multiples of 512 (psum bank boundaries)."""
    out = []
    while a < b:
        e = min(b, (a // 512 + 1) * 512)
        out.append((a, e))
        a = e
    return out


def _emit_pass(nc, psp, N, lhsT_of, wsb, sink, copy_engines):
    """One DWT pass: for each m-chunk, banded matmuls into two psum tiles
    (independent lo/hi pipelining lanes), then sink(ci, (lo, hi)) emits the
    copies out of psum."""
    Np = nprime(N)
    slots = SLOTS[N]
    chunks = sink.chunks
    f32 = _dt().float32
    for ci, m0 in enumerate(chunks):
        ps_lo = psp.tile([128, 515], f32, tag="psL", bufs=2,
                         name=f"psL_{sink.tag}_{ci}")
        ps_hi = psp.tile([128, 515], f32, tag="psH", bufs=2,
                         name=f"psH_{sink.tag}_{ci}")
        for f, ps in ((0, ps_lo), (1, ps_hi)):
            for si, (o, j0, j1) in enumerate(slots):
                lhsT = lhsT_of(si, m0)
                for (a, b) in _split_512(j0, j1):
                    w0 = a - j0
                    nc.tensor.matmul(ps[0:128, a:b], lhsT,
                                     wsb[:, si, f, w0:w0 + (b - a)],
                                     start=True, stop=True)
            # drain this quadrant's lane right away so its psum buffer
            # frees while the other quadrant's matmuls run
            sink(ci, f, ps, copy_engines)


def _dt():
    import concourse.mybir as mybir
    return mybir.dt


class _EngRot:
    """round-robin copy chooser, weighted toward the faster engines."""

    def __init__(self, nc):
        def act(out, in_):
            nc.scalar.copy(out=out, in_=in_)

        def dve(out, in_):
            nc.vector.tensor_copy(out=out, in_=in_)

        self.seq = [act, dve]
        self.i = 0

    def next(self):
        e = self.seq[self.i % len(self.seq)]
        self.i += 1
        return e


def build_bass(n_images=IMGS_PER_CORE, repeats=1):
    import concourse.mybir as mybir
    import concourse.tile as tile
    from concourse import bacc
    from concourse.ap import AP
    from contextlib import ExitStack

    nc = bacc.Bacc("TRN2", target_bir_lowering=False, debug=False)

    xin = nc.dram_tensor("xin", (n_images, H, W), mybir.dt.float32,
                         kind="ExternalInput").ap()
    out = nc.dram_tensor("out", (n_images, IMG_ELEMS), mybir.dt.float32,
                         kind="ExternalOutput").ap()
    wdram = {N: nc.dram_tensor(f"w{N}", WEIGHTS[N].shape, mybir.dt.float16,
                               kind="ExternalInput").ap() for N in LEVEL_NS}

    with tile.TileContext(nc) as tc, ExitStack() as ctx:
        cpool = ctx.enter_context(tc.tile_pool(name="consts", bufs=1))
        apool = ctx.enter_context(tc.tile_pool(name="act", bufs=1))
        stpool = ctx.enter_context(tc.tile_pool(name="stage", bufs=1))
        psp = ctx.enter_context(tc.tile_pool(name="ps", bufs=1, space="PSUM"))

        wsb = {}
        for N in LEVEL_NS:
            wsb[N] = cpool.tile(list(WEIGHTS[N].shape), mybir.dt.float16,
                                name=f"wsb{N}")
            nc.sync.dma_start(out=wsb[N][:], in_=wdram[N])

        rot = _EngRot(nc)
        for _rep in range(repeats):
            for img0 in range(0, n_images, 2):
                pair = [i for i in (img0, img0 + 1) if i < n_images]
                plans = [_image_passes(nc, apool, stpool, psp, wsb, xin, out,
                                       img, rot, AP) for img in pair]
                for k in range(6):
                    for p in plans:
                        p[k]()

    nc.compile()
    return nc


class _P1Sink:
    """p1: psum [128, 2Np] -> B[:, ci, :] (fp16)."""

    def __init__(self, nc, Np, Btile, tag):
        self.nc, self.Np, self.B, self.tag = nc, Np, Btile, tag
        self.chunks = [o for o, _, _ in SLOTS[{515: 1024, 261: 515,
                                               134: 261}[Np]]]

    def __call__(self, ci, f, ps, rot):
        Np = self.Np
        rot.next()(self.B[:, ci, f * Np:(f + 1) * Np], ps[:, 0:Np])


class _P2Sink:
    """p2: psum -> next-level A (fp16) + fp32 det staging tiles; DMAs are
    emitted by the caller once the stage tiles fill."""

    def __init__(self, nc, N, A_next, st_lo_hi, st_hi_lo, st_hi_hi, st_lo_lo):
        # st_lo_hi = cV stage (lo chunks, hi cols); st_hi_lo = cH;
        # st_hi_hi = cD; st_lo_lo = cA3 stage (only for the last level).
        self.nc = nc
        self.Np = nprime(N)
        self.A_next, self.cv, self.ch, self.cd = A_next, st_lo_hi, st_hi_lo, st_hi_hi
        self.ca = st_lo_lo
        self.lo = P2_LO[N]
        self.hi = P2_HI[N]
        self.chunks = self.lo + self.hi
        self.N = N

    def __call__(self, ci, f, ps, rot):
        Np = self.Np
        if ci < len(self.lo):
            if f == 0:
                dst = self.A_next if self.A_next is not None else self.ca
                rot.next()(dst[:, ci, :], ps[:, 0:Np])
            else:
                rot.next()(self.cv[:, ci, :], ps[:, 0:Np])
        else:
            hc = ci - len(self.lo)
            if f == 0:
                rot.next()(self.ch[:, hc, :], ps[:, 0:Np])
            elif not (self.N == 1024 and hc == len(self.hi) - 1):
                # cD1's last slot is fully covered by slot 3 (truncation)
                rot.next()(self.cd[:, hc, :], ps[:, 0:Np])


def _emit_det_dmas(nc, out, img, name, stage, row_offs, AP):
    """DMA a staged detail section (overlapping slot layout) to DRAM."""
    sec_base, Wd = SECT[name]
    base = img * IMG_ELEMS + sec_base
    n_slots = len(row_offs)
    if name == "cD1":
        # slots rows {0,128,256,384,387}; valid: 469 full rows + 404 cols
        dst = AP(out.tensor, base, [[Wd, 128], [128 * Wd, 3], [1, Wd]])
        nc.sync.dma_start(out=dst, in_=stage[:, 0:3, :])
        dst = AP(out.tensor, base + 384 * Wd, [[Wd, 85], [1, Wd]])
        nc.sync.dma_start(out=dst, in_=stage[0:85, 3, :])
        dst = AP(out.tensor, base + CD1_FULL_ROWS * Wd, [[1, CD1_PART_COLS]])
        nc.sync.dma_start(out=dst, in_=stage[85:86, 3, 0:CD1_PART_COLS])
        return
    if n_slots == 2 and row_offs[1] - row_offs[0] < 128:
        # one overlapping-dest DMA covers everything (L3 sections)
        step = row_offs[1] - row_offs[0]
        dst = AP(out.tensor, base, [[Wd, 128], [step * Wd, 2], [1, Wd]])
        nc.sync.dma_start(out=dst, in_=stage[:, 0:2, :])
        return
    # uniform prefix + fresh tail
    step = row_offs[1] - row_offs[0]
    nu = n_slots - 1
    dst = AP(out.tensor, base, [[Wd, 128], [step * Wd, nu], [1, Wd]])
    nc.sync.dma_start(out=dst, in_=stage[:, 0:nu, :])
    o_last = row_offs[-1]
    fresh0 = row_offs[-2] + 128          # first row not covered by prefix
    p0 = fresh0 - o_last
    npart = o_last + 128 - fresh0
    dst = AP(out.tensor, base + fresh0 * Wd, [[Wd, npart], [1, Wd]])
    nc.sync.dma_start(out=dst, in_=stage[p0:p0 + npart, n_slots - 1, :])


def _image_passes(nc, apool, stpool, psp, wsb, xin, out, img, rot, AP):
    """Allocate the image's tiles, emit its input DMAs, and return the six
    pass thunks (p1a, p2a, p1b, p2b, p1c, p2c) for pairwise interleaving."""
    import concourse.mybir as mybir
    f16, f32 = mybir.dt.float16, mybir.dt.float32

    # input: cast DMA into overlapping h-slots
    A1 = apool.tile([128, 9, 1024], f16, tag="A1", bufs=2, name=f"A1_{img}")
    src = AP(xin.tensor, img * H * W, [[W, 128], [122 * W, 8], [1, W]])
    nc.gpsimd.dma_start(out=A1[:, 0:8, :], in_=src)
    nc.gpsimd.dma_start(out=A1[:, 8, :], in_=xin[img, 896:1024, :])

    B1 = apool.tile([128, 9, 1030], f16, tag="B1", bufs=2, name=f"B1_{img}")
    A2 = apool.tile([128, 5, 515], f16, tag="A2", bufs=2, name=f"A2_{img}")
    B2 = apool.tile([128, 5, 522], f16, tag="B2", bufs=2, name=f"B2_{img}")
    A3 = apool.tile([128, 3, 261], f16, tag="A3", bufs=2, name=f"A3_{img}")
    B3 = apool.tile([128, 3, 268], f16, tag="B3", bufs=2, name=f"B3_{img}")

    def p1a():
        s = _P1Sink(nc, 515, B1, f"p1a_{img}")
        _emit_pass(nc, psp, 1024, lambda si, m0: A1[:, si, m0:m0 + 128],
                   wsb[1024], s, rot)

    def p2a():
        cv1 = stpool.tile([128, 5, 515], f32, tag="st515", bufs=6,
                          name=f"cv1_{img}")
        ch1 = stpool.tile([128, 5, 515], f32, tag="st515", bufs=6,
                          name=f"ch1_{img}")
        cd1 = stpool.tile([128, 5, 515], f32, tag="st515", bufs=6,
                          name=f"cd1_{img}")
        s = _P2Sink(nc, 1024, A2, cv1, ch1, cd1, None)
        s.tag = f"p2a_{img}"
        _emit_pass(nc, psp, 1024, lambda si, m0: B1[:, si, m0:m0 + 128],
                   wsb[1024], s, rot)
        _emit_det_dmas(nc, out, img, "cV1", cv1, [0, 122, 244, 366, 387], AP)
        _emit_det_dmas(nc, out, img, "cH1", ch1, [0, 128, 256, 384, 387], AP)
        _emit_det_dmas(nc, out, img, "cD1", cd1, [0, 128, 256, 384, 387], AP)

    def p1b():
        s = _P1Sink(nc, 261, B2, f"p1b_{img}")
        _emit_pass(nc, psp, 515, lambda si, m0: A2[:, si, m0:m0 + 128],
                   wsb[515], s, rot)

    def p2b():
        cv2 = stpool.tile([128, 3, 261], f32, tag="st261", bufs=6,
                          name=f"cv2_{img}")
        ch2 = stpool.tile([128, 3, 261], f32, tag="st261", bufs=6,
                          name=f"ch2_{img}")
        cd2 = stpool.tile([128, 3, 261], f32, tag="st261", bufs=6,
                          name=f"cd2_{img}")
        s = _P2Sink(nc, 515, A3, cv2, ch2, cd2, None)
        s.tag = f"p2b_{img}"
        _emit_pass(nc, psp, 515, lambda si, m0: B2[:, si, m0:m0 + 128],
                   wsb[515], s, rot)
        _emit_det_dmas(nc, out, img, "cV2", cv2, [0, 122, 133], AP)
        _emit_det_dmas(nc, out, img, "cH2", ch2, [0, 128, 133], AP)
        _emit_det_dmas(nc, out, img, "cD2", cd2, [0, 128, 133], AP)

    def p1c():
        s = _P1Sink(nc, 134, B3, f"p1c_{img}")
        _emit_pass(nc, psp, 261, lambda si, m0: A3[:, si, m0:m0 + 128],
                   wsb[261], s, rot)

    def p2c():
        ca3 = stpool.tile([128, 2, 134], f32, tag="st134", bufs=8,
                          name=f"ca3_{img}")
        cv3 = stpool.tile([128, 2, 134], f32, tag="st134", bufs=8,
                          name=f"cv3_{img}")
        ch3 = stpool.tile([128, 2, 134], f32, tag="st134", bufs=8,
                          name=f"ch3_{img}")
        cd3 = stpool.tile([128, 2, 134], f32, tag="st134", bufs=8,
                          name=f"cd3_{img}")
        s = _P2Sink(nc, 261, None, cv3, ch3, cd3, ca3)
        s.tag = f"p2c_{img}"
        _emit_pass(nc, psp, 261, lambda si, m0: B3[:, si, m0:m0 + 128],
                   wsb[261], s, rot)
        _emit_det_dmas(nc, out, img, "cA3", ca3, [0, 6], AP)
        _emit_det_dmas(nc, out, img, "cV3", cv3, [0, 6], AP)
        _emit_det_dmas(nc, out, img, "cH3", ch3, [0, 6], AP)
        _emit_det_dmas(nc, out, img, "cD3", cd3, [0, 6], AP)

    return [p1a, p2a, p1b, p2b, p1c, p2c]


# ----------------------------------------------------------------- runner
EXTRA_INPUTS = {f"w{N}": WEIGHTS[N] for N in LEVEL_NS}


def _get_built():
    global _BUILT
    if _BUILT is None:
        _BUILT = build_bass()
    return _BUILT


def kernel(x: np.ndarray) -> np.ndarray:
    from concourse import bass_utils

    x = np.ascontiguousarray(np.asarray(x), dtype=np.float32)
    assert x.shape == (B, C, H, W), x.shape
    nc = _get_built()

    imgs = x.reshape(B * C, H, W)
    in_maps = []
    for c in range(N_CORES):
        m = {"xin": imgs[c * IMGS_PER_CORE:(c + 1) * IMGS_PER_CORE]}
        m.update(EXTRA_INPUTS)
        in_maps.append(m)

    res = bass_utils.run_bass_kernel_spmd(nc, in_maps,
                                          core_ids=list(range(N_CORES)))
    outs = [res.results[c]["out"] for c in range(N_CORES)]
    flat = np.concatenate(outs, axis=0)  # [48, 1048576]
    return flat.reshape(B, C, 64, 128, 128)
